# revision 21
# baseline (speedup 1.0000x reference)
"""Ensemble detection fusion (weighted-boxes-fusion match + soft-NMS dedup)
for Trainium2, 8 NeuronCores.

Strategy: the O(N^2) work — greedy-match IoU tests (yolo x frcnn @ 0.8) and
NMS IoU tests (all x all @ 0.95) — runs on-device as ONE merged conservative
*filter* sweep, sharded row-wise across the 8 cores.  Matches/suppressions
at these thresholds are extremely sparse, so the host then *rescues* only
the flagged rows with the exact reference arithmetic (fp32, matching op
order) and resolves the short sequential dependency chains (greedy 'used'
set, NMS suppression scan) on those few rows.

Device test per pair (q, t), fp16 pixel space, split across DVE (ops in
their fast perf modes: tensor_scalar 4x, tensor_tensor 2x) and the
otherwise-idle Scalar engine (relu clamps + sign + count accumulation):
    dxpk = min(TX2,qx2) + min(K-TX1, K-qx1)          # = dx + K     (DVE)
    dypk = min(TY2,qy2) + min(K-TY1, K-qy1)          # = dy + K     (DVE)
    dxk, dyk = relu(dxpk), relu(dypk)                               (ACT)
    v    = dxk*dyk - TS_t                                           (DVE)
    cnt  = sum_j sign(v_j - QS_q)                                   (ACT)
with TS/QS = c*(1-MU)*area, c = thr/(1+thr).  In exact arithmetic
inter >= c*(Aq+At) <=> iou >= thr; the +K (3 px) additive slack covers
fp16 coordinate rounding (~1px absolute on dx/dy) and MU covers relative
rounding, so the device pass set is a strict superset of the exact set.
Per-row pass count is recovered from the sign-sum as P = (cnt + C)/2
(C = swept columns); exact-zero ties only ADD margin-zone false positives.

Work layout: the NMS matrix is symmetric, so rows sweep only
j >= 1024*floor(i/1024) (block upper triangle); rows are assigned to cores
round-robin (i mod 8) so each core holds one 128-row tile per 1024-row
block and the triangle is perfectly balanced.  Every pair (i,j), i<j, is
swept by row i.  For yolo rows (i < 4096) the frcnn column range
[4096:6144) is swept with the LOOSER greedy-match threshold (c_A < c_B),
which simultaneously provides the stage-A candidate set and a superset of
the stage-B edges in that region — stage A costs no extra sweep.
"""

import numpy as np

N1, N2 = 4096, 2048
NB = N1 + N2
CORES = 8
YOLO_W = 0.5
FRCNN_W = 0.5
MATCH_IOU = 0.8
NMS_IOU = 0.95
MU = 0.02  # relative margin on the device filter
KPX = 3.0  # additive pixel slack on overlap widths

QB_PT = NB // CORES // 128  # 6 query tiles per core

# chunk plan: (qtile k, target start, size, area-scale selector)
# sel 'B' -> c_B areas (NMS 0.95), sel 'A' -> c_A areas (match 0.8).
# qtile k of core c holds global rows {1024*k + c + 8*m} and sweeps
# targets j >= 1024*k.
PLAN = [
    # ordered so fresh broadcast ranges are needed as late as possible:
    # k0 warms up on r0/r1, k1-k3 reuse delivered ranges, the stage-A
    # chunks need only the small SA block, and k4/k5 (fresh r4-r5) run
    # last so their delivery hides under the stage-A compute
    (0, 0, 1024, "B"),
    (0, 1024, 1024, "B"),
    (0, 2048, 2048, "B"),
    (1, 1024, 3072, "B"),
    (2, 2048, 2048, "B"),
    (3, 3072, 1024, "B"),
    (0, 4096, 2048, "A"),
    (1, 4096, 2048, "A"),
    (2, 4096, 2048, "A"),
    (3, 4096, 2048, "A"),
    (4, 4096, 2048, "B"),
    (5, 5120, 1024, "B"),
]
# columns swept per qtile k (both selectors)
CB_K = [6144 - 1024 * k for k in range(QB_PT)]

# target blob (fp16): X2[NB] Y2[NB] NX1K[NB] NY1K[NB] SB[NB] SA[N2]
# where NX1K = KPX - x1, NY1K = KPX - y1, SB = cB*(1-MU)*area (all boxes),
# SA = cA*(1-MU)*area (frcnn boxes, for the stage-A columns).
TLEN = 5 * NB + N2

_PROGRAM_CACHE = {}


def _emit_combo(nc, mybir, X2, Y2, NX1K, NY1K, TS, q, nqs, pw, pc, out_ap, chunk):
    f16 = mybir.dt.float16
    f32 = mybir.dt.float32
    Alu = mybir.AluOpType
    Act = mybir.ActivationFunctionType
    nqx1k = q[:, 0:1]
    nqy1k = q[:, 1:2]
    qx2 = q[:, 2:3]
    qy2 = q[:, 3:4]
    m1x = pw.tile([128, chunk], f16, tag="m1")
    nc.vector.tensor_scalar(m1x[:, :], X2, qx2, None, Alu.min)
    a1x = pw.tile([128, chunk], f16, tag="a1")
    nc.vector.tensor_scalar(a1x[:, :], NX1K, nqx1k, None, Alu.min)
    dxpk = pw.tile([128, chunk], f16, tag="dp")
    nc.vector.tensor_tensor(dxpk[:, :], m1x[:, :], a1x[:, :], Alu.add)
    m1y = pw.tile([128, chunk], f16, tag="m1")
    nc.vector.tensor_scalar(m1y[:, :], Y2, qy2, None, Alu.min)
    a1y = pw.tile([128, chunk], f16, tag="a1")
    nc.vector.tensor_scalar(a1y[:, :], NY1K, nqy1k, None, Alu.min)
    dypk = pw.tile([128, chunk], f16, tag="dp2")
    nc.vector.tensor_tensor(dypk[:, :], m1y[:, :], a1y[:, :], Alu.add)
    dxk = pw.tile([128, chunk], f16, tag="dk")
    nc.scalar.activation(dxk[:, :], dxpk[:, :], Act.Relu)
    dyk = pw.tile([128, chunk], f16, tag="dk2")
    nc.scalar.activation(dyk[:, :], dypk[:, :], Act.Relu)
    p = pw.tile([128, chunk], f16, tag="p")
    nc.vector.tensor_tensor(p[:, :], dxk[:, :], dyk[:, :], Alu.mult)
    v = pw.tile([128, chunk], f16, tag="v")
    nc.vector.tensor_tensor(v[:, :], p[:, :], TS, Alu.subtract)
    g = pw.tile([128, chunk], f16, tag="g")
    cnt = pc.tile([128, 1], f32, tag="cnt")
    nc.scalar.activation(
        g[:, :], v[:, :], Act.Sign, bias=nqs, accum_out=cnt[:, :]
    )
    nc.sync.dma_start(out_ap, cnt[:, :])


def _build_program():
    import concourse.bacc as bacc
    import concourse.mybir as mybir
    from concourse import tile

    f16 = mybir.dt.float16
    f32 = mybir.dt.float32
    nc = bacc.Bacc(
        "TRN2", target_bir_lowering=False, debug=False, num_devices=CORES
    )
    qb = nc.dram_tensor("qb", [QB_PT, 128, 6], f32, kind="ExternalInput")
    tbl = nc.dram_tensor("tbl", [1, TLEN], f16, kind="ExternalInput")
    outc = nc.dram_tensor("outc", [len(PLAN), 128, 1], f32, kind="ExternalOutput")

    with tile.TileContext(nc) as tc:
        with (
            tc.tile_pool(name="tgt", bufs=1) as pt,
            tc.tile_pool(name="qs", bufs=8) as pq,
            tc.tile_pool(name="wk", bufs=2) as pw,
            tc.tile_pool(name="ct", bufs=6) as pc,
        ):
            tfull = pt.tile([128, TLEN], f16, tag="tfull")

            # replicate the target blob across all 128 partitions with
            # stride-0-source DMAs on the sync queue (no GpSimd engine
            # involvement -> no shared-SBUF-port contention with DVE).
            # Pieces are emitted on demand just before the first combo
            # that reads them, so each combo's semaphore wait gates on an
            # early cumulative DMA count instead of the whole broadcast.
            done_ranges = set()

            bcast_engines = [nc.sync, nc.scalar]
            bcast_i = [0]

            def bcast(off, n):
                eng = bcast_engines[bcast_i[0] % 2]
                bcast_i[0] += 1
                eng.dma_start(
                    tfull[:, off : off + n],
                    tbl.ap()[0:1, off : off + n].partition_broadcast(128),
                )

            def need_cols(start, size, sel):
                for r in range(start // 1024, (start + size + 1023) // 1024):
                    for b in range(5):  # X2 Y2 NX1K NY1K SB blocks
                        key = (b, r)
                        if key not in done_ranges:
                            done_ranges.add(key)
                            bcast(b * NB + r * 1024, 1024)
                if sel == "A":
                    for r in range(2):
                        key = (5, r)
                        if key not in done_ranges:
                            done_ranges.add(key)
                            bcast(5 * NB + r * 1024, 1024)

            X2 = tfull[:, 0 * NB : 1 * NB]
            Y2 = tfull[:, 1 * NB : 2 * NB]
            NX1K = tfull[:, 2 * NB : 3 * NB]
            NY1K = tfull[:, 3 * NB : 4 * NB]
            SB = tfull[:, 4 * NB : 5 * NB]
            SA = tfull[:, 5 * NB : 5 * NB + N2]

            qtiles = {}
            for idx, (k, start, size, sel) in enumerate(PLAN):
                if k not in qtiles:
                    q = pq.tile([128, 6], f32, tag="q")
                    nc.sync.dma_start(q[:, :], qb.ap()[k, :, :])
                    qtiles[k] = q
                q = qtiles[k]
                need_cols(start, size, sel)
                sl = slice(start, start + size)
                ts_ap = SB[:, sl] if sel == "B" else SA[:, start - N1 : start - N1 + size]
                nqs = q[:, 4:5] if sel == "B" else q[:, 5:6]
                _emit_combo(
                    nc, mybir,
                    X2[:, sl], Y2[:, sl], NX1K[:, sl], NY1K[:, sl], ts_ap,
                    q, nqs, pw, pc, outc.ap()[idx, :, :], size,
                )
    nc.compile()
    return nc


def get_program():
    if "nc" not in _PROGRAM_CACHE:
        _PROGRAM_CACHE["nc"] = _build_program()
    return _PROGRAM_CACHE["nc"]


def _iou_row(box, B):
    # Exact replica of reference _iou_one_vs_many op order (fp32, IEEE).
    x1 = np.maximum(box[0], B[:, 0])
    y1 = np.maximum(box[1], B[:, 1])
    x2 = np.minimum(box[2], B[:, 2])
    y2 = np.minimum(box[3], B[:, 3])
    inter = np.maximum(x2 - x1, np.float32(0.0)) * np.maximum(y2 - y1, np.float32(0.0))
    a1 = (box[2] - box[0]) * (box[3] - box[1])
    a2 = (B[:, 2] - B[:, 0]) * (B[:, 3] - B[:, 1])
    return inter / (a1 + a2 - inter)


def _stage_b_rows(core):
    """Global row indices handled by `core`, tile-major: [k, m] -> row."""
    k = np.arange(QB_PT)[:, None]
    m = np.arange(128)[None, :]
    return 1024 * k + core + 8 * m


def make_device_inputs(pall):
    """pall: pixel-space fp32 box array, yolo rows then frcnn rows."""
    aall = (pall[:, 2] - pall[:, 0]) * (pall[:, 3] - pall[:, 1])
    cA = np.float32((1.0 - MU) * MATCH_IOU / (1.0 + MATCH_IOU))
    cB = np.float32((1.0 - MU) * NMS_IOU / (1.0 + NMS_IOU))
    kpx = np.float32(KPX)

    # query cols: K-x1, K-y1, x2, y2, -cB*A, -cA*A (ACT bias adds before Sign)
    qb_all = np.stack(
        [kpx - pall[:, 0], kpx - pall[:, 1], pall[:, 2], pall[:, 3],
         -cB * aall, -cA * aall], axis=1
    ).astype(np.float32)
    tbl = np.concatenate(
        [pall[:, 2], pall[:, 3], kpx - pall[:, 0], kpx - pall[:, 1],
         cB * aall, cA * aall[N1:]]
    ).astype(np.float16).reshape(1, -1)
    tbl = np.ascontiguousarray(tbl)
    in_maps = []
    for c in range(CORES):
        rows = _stage_b_rows(c).reshape(-1)
        in_maps.append(
            {
                "qb": np.ascontiguousarray(qb_all[rows].reshape(QB_PT, 128, 6)),
                "tbl": tbl,
            }
        )
    return in_maps


def kernel(**inputs):
    yolo_boxes = np.asarray(inputs["yolo_boxes"], dtype=np.float32)
    yolo_scores = np.asarray(inputs["yolo_scores"], dtype=np.float32)
    yolo_labels = np.asarray(inputs["yolo_labels"], dtype=np.int32)
    frcnn_boxes = np.asarray(inputs["frcnn_boxes"], dtype=np.float32)
    frcnn_scores = np.asarray(inputs["frcnn_scores"], dtype=np.float32)
    frcnn_labels = np.asarray(inputs["frcnn_labels"], dtype=np.int32)
    h = float(np.asarray(inputs["h"]))
    w = float(np.asarray(inputs["w"]))

    wh = np.array([w, h, w, h], dtype=np.float32)
    b1 = (yolo_boxes / wh).astype(np.float32)
    b2 = (frcnn_boxes / wh).astype(np.float32)
    s1 = (yolo_scores * np.float32(YOLO_W)).astype(np.float32)
    s2 = (frcnn_scores * np.float32(FRCNN_W)).astype(np.float32)
    l1, l2 = yolo_labels, frcnn_labels

    # --- device filter: 8-core SPMD merged IoU-test sweep ---
    from concourse.bass_utils import run_bass_kernel_spmd

    nc = get_program()
    pall = np.concatenate([yolo_boxes, frcnn_boxes], axis=0)
    in_maps = make_device_inputs(pall)
    import time as _time

    _t0 = _time.time()
    res = run_bass_kernel_spmd(nc, in_maps, core_ids=list(range(CORES)))
    _PROGRAM_CACHE["device_wall_ns"] = int((_time.time() - _t0) * 1e9)
    if getattr(res, "exec_time_ns", None) is not None:
        _PROGRAM_CACHE["exec_time_ns"] = res.exec_time_ns

    # sign-sums -> pass counts: P = (sum + C)/2
    sig_all = np.zeros(NB, dtype=np.float64)   # all chunks (flags_b)
    sig_a = np.zeros(NB, dtype=np.float64)     # cA chunks only (flags_a)
    for c in range(CORES):
        rows = _stage_b_rows(c)  # [QB_PT, 128]
        oc = res.results[c]["outc"].reshape(len(PLAN), 128)
        for idx, (k, _start, _size, sel) in enumerate(PLAN):
            sig_all[rows[k]] += oc[idx]
            if sel == "A":
                sig_a[rows[k]] += oc[idx]

    pass_a = (sig_a[:N1] + N2) * 0.5
    flags_a = np.nonzero(pass_a > 0.5)[0]

    cb = np.zeros(NB)
    for k in range(QB_PT):
        cb[1024 * k : 1024 * (k + 1)] = CB_K[k]
    pass_b = (sig_all + cb) * 0.5
    # every row sweeps itself (diagonal), so > 1 means a real
    # (off-diagonal) near-duplicate candidate
    flags_b = np.nonzero(pass_b > 1.5)[0]

    return _host_resolve(b1, b2, s1, s2, l1, l2, wh, flags_a, flags_b)


def _host_resolve(b1, b2, s1, s2, l1, l2, wh, flags_a, flags_b):
    # --- exact greedy match on candidate rows only ---
    used = np.zeros(N2, dtype=bool)
    mboxes = b1.copy()
    mscores = s1.copy()
    merged_rows = []
    thr_match = np.float32(MATCH_IOU)
    for i in flags_a:
        iou = _iou_row(b1[i], b2)
        iou = np.where((~used) & (l2 == l1[i]), iou, np.float32(0.0))
        j = int(np.argmax(iou))
        if iou[j] >= thr_match:
            tot = s1[i] + s2[j]
            mboxes[i] = (b1[i] * s1[i] + b2[j] * s2[j]) / tot
            mscores[i] = tot
            used[j] = True
            merged_rows.append(int(i))

    boxes_all = np.concatenate([mboxes, b2], axis=0)
    scores_all = np.concatenate([mscores, s2], axis=0)
    labels_all = np.concatenate([l1, l2], axis=0)
    valid = np.concatenate([np.ones(N1, dtype=bool), ~used])

    key = np.where(valid, scores_all, np.float32(-1.0))
    order = np.argsort(-key, kind="stable")
    bs = boxes_all[order]
    ss = scores_all[order]
    ls = labels_all[order]
    vs = valid[order]
    inv = np.empty(NB, dtype=np.int64)
    inv[order] = np.arange(NB)

    # --- exact NMS edge rescue ---
    # device sweep used PRE-merge boxes: flagged rows cover all edges
    # between unmerged boxes; merged rows are rescanned fully.
    r_unsorted = set(int(x) for x in flags_b) | set(merged_rows)
    thr_nms = np.float32(NMS_IOU)
    edges = {}
    for r in r_unsorted:
        p = int(inv[r])
        iou = _iou_row(bs[p], bs)
        hits = np.nonzero((iou >= thr_nms) & (ls == ls[p]))[0]
        for qq in hits:
            qq = int(qq)
            if qq == p:
                continue
            lo, hi = (p, qq) if p < qq else (qq, p)
            edges.setdefault(lo, set()).add(hi)

    # --- sequential suppression scan (only edge nodes matter) ---
    suppressed = ~vs
    keep = np.zeros(NB, dtype=bool)
    if edges:
        for i in range(NB):
            k = not suppressed[i]
            keep[i] = k
            if k and i in edges:
                for b_ in edges[i]:
                    suppressed[b_] = True
    else:
        keep = vs.copy()

    label_map = np.array([2, 1], dtype=np.int32)
    boxes_out = (bs * wh).astype(np.float32)
    labels_out = label_map[np.clip(ls, 0, 1)].astype(np.int32)
    return boxes_out, labels_out, ss.astype(np.float32), keep


# revision 22
# speedup vs baseline: 1.0519x; 1.0519x over previous
"""Ensemble detection fusion (weighted-boxes-fusion match + soft-NMS dedup)
for Trainium2, 8 NeuronCores.

Strategy: the O(N^2) work — greedy-match IoU tests (yolo x frcnn @ 0.8) and
NMS IoU tests (all x all @ 0.95) — runs on-device as ONE merged conservative
*filter* sweep, sharded row-wise across the 8 cores.  Matches/suppressions
at these thresholds are extremely sparse, so the host then *rescues* only
the flagged rows with the exact reference arithmetic (fp32, matching op
order) and resolves the short sequential dependency chains (greedy 'used'
set, NMS suppression scan) on those few rows.

Device test per pair (q, t), fp16 pixel space, split across DVE (ops in
their fast perf modes: tensor_scalar 4x, tensor_tensor 2x) and the
otherwise-idle Scalar engine (relu clamps + sign + count accumulation):
    dxpk = min(TX2,qx2) + min(K-TX1, K-qx1)          # = dx + K     (DVE)
    dypk = min(TY2,qy2) + min(K-TY1, K-qy1)          # = dy + K     (DVE)
    dxk, dyk = relu(dxpk), relu(dypk)                               (ACT)
    v    = dxk*dyk - TS_t                                           (DVE)
    cnt  = sum_j sign(v_j - QS_q)                                   (ACT)
with TS/QS = c*(1-MU)*area, c = thr/(1+thr).  In exact arithmetic
inter >= c*(Aq+At) <=> iou >= thr; the +K (3 px) additive slack covers
fp16 coordinate rounding (~1px absolute on dx/dy) and MU covers relative
rounding, so the device pass set is a strict superset of the exact set.
Per-row pass count is recovered from the sign-sum as P = (cnt + C)/2
(C = swept columns); exact-zero ties only ADD margin-zone false positives.

Work layout: the NMS matrix is symmetric, so rows sweep only
j >= 1024*floor(i/1024) (block upper triangle); rows are assigned to cores
round-robin (i mod 8) so each core holds one 128-row tile per 1024-row
block and the triangle is perfectly balanced.  Every pair (i,j), i<j, is
swept by row i.  For yolo rows (i < 4096) the frcnn column range
[4096:6144) is swept with the LOOSER greedy-match threshold (c_A < c_B),
which simultaneously provides the stage-A candidate set and a superset of
the stage-B edges in that region — stage A costs no extra sweep.
"""

import numpy as np

N1, N2 = 4096, 2048
NB = N1 + N2
CORES = 8
YOLO_W = 0.5
FRCNN_W = 0.5
MATCH_IOU = 0.8
NMS_IOU = 0.95
MU = 0.02  # relative margin on the device filter
KPX = 3.0  # additive pixel slack on overlap widths

QB_PT = NB // CORES // 128  # 6 query tiles per core

# chunk plan: (qtile k, target start, size, area-scale selector)
# sel 'B' -> c_B areas (NMS 0.95), sel 'A' -> c_A areas (match 0.8).
# qtile k of core c holds global rows {1024*k + c + 8*m} and sweeps
# targets j >= 1024*k.
PLAN = [
    # ordered so combos needing only early blob ranges run first, the
    # stage-A (SA-dependent) chunks late, and a short chunk last so the
    # final ACT/drain tail is small
    (0, 0, 1024, "B"),
    (0, 1024, 3072, "B"),
    (1, 1024, 3072, "B"),
    (2, 2048, 2048, "B"),
    (4, 4096, 2048, "B"),
    (5, 5120, 1024, "B"),
    (0, 4096, 2048, "A"),
    (1, 4096, 2048, "A"),
    (2, 4096, 2048, "A"),
    (3, 4096, 2048, "A"),
    (3, 3072, 1024, "B"),
]
# columns swept per qtile k (both selectors)
CB_K = [6144 - 1024 * k for k in range(QB_PT)]

# target blob (fp16): X2[NB] Y2[NB] NX1K[NB] NY1K[NB] SB[NB] SA[N2]
# where NX1K = KPX - x1, NY1K = KPX - y1, SB = cB*(1-MU)*area (all boxes),
# SA = cA*(1-MU)*area (frcnn boxes, for the stage-A columns).
TLEN = 5 * NB + N2

_PROGRAM_CACHE = {}


def _emit_combo(nc, mybir, X2, Y2, NX1K, NY1K, TS, q, nqs, pw, pc, out_ap, chunk):
    f16 = mybir.dt.float16
    f32 = mybir.dt.float32
    Alu = mybir.AluOpType
    Act = mybir.ActivationFunctionType
    nqx1k = q[:, 0:1]
    nqy1k = q[:, 1:2]
    qx2 = q[:, 2:3]
    qy2 = q[:, 3:4]
    m1x = pw.tile([128, chunk], f16, tag="m1")
    nc.vector.tensor_scalar(m1x[:, :], X2, qx2, None, Alu.min)
    a1x = pw.tile([128, chunk], f16, tag="a1")
    nc.vector.tensor_scalar(a1x[:, :], NX1K, nqx1k, None, Alu.min)
    dxpk = pw.tile([128, chunk], f16, tag="dp")
    nc.vector.tensor_tensor(dxpk[:, :], m1x[:, :], a1x[:, :], Alu.add)
    m1y = pw.tile([128, chunk], f16, tag="m1")
    nc.vector.tensor_scalar(m1y[:, :], Y2, qy2, None, Alu.min)
    a1y = pw.tile([128, chunk], f16, tag="a1")
    nc.vector.tensor_scalar(a1y[:, :], NY1K, nqy1k, None, Alu.min)
    dypk = pw.tile([128, chunk], f16, tag="dp2")
    nc.vector.tensor_tensor(dypk[:, :], m1y[:, :], a1y[:, :], Alu.add)
    dxk = pw.tile([128, chunk], f16, tag="dk")
    nc.scalar.activation(dxk[:, :], dxpk[:, :], Act.Relu)
    dyk = pw.tile([128, chunk], f16, tag="dk2")
    nc.scalar.activation(dyk[:, :], dypk[:, :], Act.Relu)
    p = pw.tile([128, chunk], f16, tag="p")
    nc.vector.tensor_tensor(p[:, :], dxk[:, :], dyk[:, :], Alu.mult)
    v = pw.tile([128, chunk], f16, tag="v")
    nc.vector.tensor_tensor(v[:, :], p[:, :], TS, Alu.subtract)
    g = pw.tile([128, chunk], f16, tag="g")
    cnt = pc.tile([128, 1], f32, tag="cnt")
    nc.scalar.activation(
        g[:, :], v[:, :], Act.Sign, bias=nqs, accum_out=cnt[:, :]
    )
    nc.sync.dma_start(out_ap, cnt[:, :])


def _build_program():
    import concourse.bacc as bacc
    import concourse.mybir as mybir
    from concourse import tile

    f16 = mybir.dt.float16
    f32 = mybir.dt.float32
    nc = bacc.Bacc(
        "TRN2", target_bir_lowering=False, debug=False, num_devices=CORES
    )
    qb = nc.dram_tensor("qb", [QB_PT, 128, 6], f32, kind="ExternalInput")
    tbl = nc.dram_tensor("tbl", [1, TLEN], f16, kind="ExternalInput")
    outc = nc.dram_tensor("outc", [len(PLAN), 128, 1], f32, kind="ExternalOutput")

    with tile.TileContext(nc) as tc:
        with (
            tc.tile_pool(name="tgt", bufs=1) as pt,
            tc.tile_pool(name="qs", bufs=8) as pq,
            tc.tile_pool(name="wk", bufs=2) as pw,
            tc.tile_pool(name="ct", bufs=6) as pc,
        ):
            tfull = pt.tile([128, TLEN], f16, tag="tfull")

            # replicate the target blob across all 128 partitions with
            # stride-0-source DMAs on the sync queue (no GpSimd engine
            # involvement -> no shared-SBUF-port contention with DVE).
            # Pieces are emitted on demand just before the first combo
            # that reads them, so each combo's semaphore wait gates on an
            # early cumulative DMA count instead of the whole broadcast.
            done_ranges = set()

            bcast_engines = [nc.sync, nc.scalar]
            bcast_i = [0]

            def bcast(off, n):
                eng = bcast_engines[bcast_i[0] % 2]
                bcast_i[0] += 1
                eng.dma_start(
                    tfull[:, off : off + n],
                    tbl.ap()[0:1, off : off + n].partition_broadcast(128),
                )

            def need_cols(start, size, sel):
                for r in range(start // 1024, (start + size + 1023) // 1024):
                    for b in range(5):  # X2 Y2 NX1K NY1K SB blocks
                        key = (b, r)
                        if key not in done_ranges:
                            done_ranges.add(key)
                            bcast(b * NB + r * 1024, 1024)
                if sel == "A":
                    for r in range(2):
                        key = (5, r)
                        if key not in done_ranges:
                            done_ranges.add(key)
                            bcast(5 * NB + r * 1024, 1024)

            X2 = tfull[:, 0 * NB : 1 * NB]
            Y2 = tfull[:, 1 * NB : 2 * NB]
            NX1K = tfull[:, 2 * NB : 3 * NB]
            NY1K = tfull[:, 3 * NB : 4 * NB]
            SB = tfull[:, 4 * NB : 5 * NB]
            SA = tfull[:, 5 * NB : 5 * NB + N2]

            qtiles = {}
            for idx, (k, start, size, sel) in enumerate(PLAN):
                if k not in qtiles:
                    q = pq.tile([128, 6], f32, tag="q")
                    nc.sync.dma_start(q[:, :], qb.ap()[k, :, :])
                    qtiles[k] = q
                q = qtiles[k]
                need_cols(start, size, sel)
                sl = slice(start, start + size)
                ts_ap = SB[:, sl] if sel == "B" else SA[:, start - N1 : start - N1 + size]
                nqs = q[:, 4:5] if sel == "B" else q[:, 5:6]
                _emit_combo(
                    nc, mybir,
                    X2[:, sl], Y2[:, sl], NX1K[:, sl], NY1K[:, sl], ts_ap,
                    q, nqs, pw, pc, outc.ap()[idx, :, :], size,
                )
    nc.compile()
    return nc


def get_program():
    if "nc" not in _PROGRAM_CACHE:
        _PROGRAM_CACHE["nc"] = _build_program()
    return _PROGRAM_CACHE["nc"]


def _iou_row(box, B):
    # Exact replica of reference _iou_one_vs_many op order (fp32, IEEE).
    x1 = np.maximum(box[0], B[:, 0])
    y1 = np.maximum(box[1], B[:, 1])
    x2 = np.minimum(box[2], B[:, 2])
    y2 = np.minimum(box[3], B[:, 3])
    inter = np.maximum(x2 - x1, np.float32(0.0)) * np.maximum(y2 - y1, np.float32(0.0))
    a1 = (box[2] - box[0]) * (box[3] - box[1])
    a2 = (B[:, 2] - B[:, 0]) * (B[:, 3] - B[:, 1])
    return inter / (a1 + a2 - inter)


def _stage_b_rows(core):
    """Global row indices handled by `core`, tile-major: [k, m] -> row."""
    k = np.arange(QB_PT)[:, None]
    m = np.arange(128)[None, :]
    return 1024 * k + core + 8 * m


def make_device_inputs(pall):
    """pall: pixel-space fp32 box array, yolo rows then frcnn rows."""
    aall = (pall[:, 2] - pall[:, 0]) * (pall[:, 3] - pall[:, 1])
    cA = np.float32((1.0 - MU) * MATCH_IOU / (1.0 + MATCH_IOU))
    cB = np.float32((1.0 - MU) * NMS_IOU / (1.0 + NMS_IOU))
    kpx = np.float32(KPX)

    # query cols: K-x1, K-y1, x2, y2, -cB*A, -cA*A (ACT bias adds before Sign)
    qb_all = np.stack(
        [kpx - pall[:, 0], kpx - pall[:, 1], pall[:, 2], pall[:, 3],
         -cB * aall, -cA * aall], axis=1
    ).astype(np.float32)
    tbl = np.concatenate(
        [pall[:, 2], pall[:, 3], kpx - pall[:, 0], kpx - pall[:, 1],
         cB * aall, cA * aall[N1:]]
    ).astype(np.float16).reshape(1, -1)
    tbl = np.ascontiguousarray(tbl)
    in_maps = []
    for c in range(CORES):
        rows = _stage_b_rows(c).reshape(-1)
        in_maps.append(
            {
                "qb": np.ascontiguousarray(qb_all[rows].reshape(QB_PT, 128, 6)),
                "tbl": tbl,
            }
        )
    return in_maps


def kernel(**inputs):
    yolo_boxes = np.asarray(inputs["yolo_boxes"], dtype=np.float32)
    yolo_scores = np.asarray(inputs["yolo_scores"], dtype=np.float32)
    yolo_labels = np.asarray(inputs["yolo_labels"], dtype=np.int32)
    frcnn_boxes = np.asarray(inputs["frcnn_boxes"], dtype=np.float32)
    frcnn_scores = np.asarray(inputs["frcnn_scores"], dtype=np.float32)
    frcnn_labels = np.asarray(inputs["frcnn_labels"], dtype=np.int32)
    h = float(np.asarray(inputs["h"]))
    w = float(np.asarray(inputs["w"]))

    wh = np.array([w, h, w, h], dtype=np.float32)
    b1 = (yolo_boxes / wh).astype(np.float32)
    b2 = (frcnn_boxes / wh).astype(np.float32)
    s1 = (yolo_scores * np.float32(YOLO_W)).astype(np.float32)
    s2 = (frcnn_scores * np.float32(FRCNN_W)).astype(np.float32)
    l1, l2 = yolo_labels, frcnn_labels

    # --- device filter: 8-core SPMD merged IoU-test sweep ---
    from concourse.bass_utils import run_bass_kernel_spmd

    nc = get_program()
    pall = np.concatenate([yolo_boxes, frcnn_boxes], axis=0)
    in_maps = make_device_inputs(pall)
    import time as _time

    _t0 = _time.time()
    res = run_bass_kernel_spmd(nc, in_maps, core_ids=list(range(CORES)))
    _PROGRAM_CACHE["device_wall_ns"] = int((_time.time() - _t0) * 1e9)
    if getattr(res, "exec_time_ns", None) is not None:
        _PROGRAM_CACHE["exec_time_ns"] = res.exec_time_ns

    # sign-sums -> pass counts: P = (sum + C)/2
    sig_all = np.zeros(NB, dtype=np.float64)   # all chunks (flags_b)
    sig_a = np.zeros(NB, dtype=np.float64)     # cA chunks only (flags_a)
    for c in range(CORES):
        rows = _stage_b_rows(c)  # [QB_PT, 128]
        oc = res.results[c]["outc"].reshape(len(PLAN), 128)
        for idx, (k, _start, _size, sel) in enumerate(PLAN):
            sig_all[rows[k]] += oc[idx]
            if sel == "A":
                sig_a[rows[k]] += oc[idx]

    pass_a = (sig_a[:N1] + N2) * 0.5
    flags_a = np.nonzero(pass_a > 0.5)[0]

    cb = np.zeros(NB)
    for k in range(QB_PT):
        cb[1024 * k : 1024 * (k + 1)] = CB_K[k]
    pass_b = (sig_all + cb) * 0.5
    # every row sweeps itself (diagonal), so > 1 means a real
    # (off-diagonal) near-duplicate candidate
    flags_b = np.nonzero(pass_b > 1.5)[0]

    return _host_resolve(b1, b2, s1, s2, l1, l2, wh, flags_a, flags_b)


def _host_resolve(b1, b2, s1, s2, l1, l2, wh, flags_a, flags_b):
    # --- exact greedy match on candidate rows only ---
    used = np.zeros(N2, dtype=bool)
    mboxes = b1.copy()
    mscores = s1.copy()
    merged_rows = []
    thr_match = np.float32(MATCH_IOU)
    for i in flags_a:
        iou = _iou_row(b1[i], b2)
        iou = np.where((~used) & (l2 == l1[i]), iou, np.float32(0.0))
        j = int(np.argmax(iou))
        if iou[j] >= thr_match:
            tot = s1[i] + s2[j]
            mboxes[i] = (b1[i] * s1[i] + b2[j] * s2[j]) / tot
            mscores[i] = tot
            used[j] = True
            merged_rows.append(int(i))

    boxes_all = np.concatenate([mboxes, b2], axis=0)
    scores_all = np.concatenate([mscores, s2], axis=0)
    labels_all = np.concatenate([l1, l2], axis=0)
    valid = np.concatenate([np.ones(N1, dtype=bool), ~used])

    key = np.where(valid, scores_all, np.float32(-1.0))
    order = np.argsort(-key, kind="stable")
    bs = boxes_all[order]
    ss = scores_all[order]
    ls = labels_all[order]
    vs = valid[order]
    inv = np.empty(NB, dtype=np.int64)
    inv[order] = np.arange(NB)

    # --- exact NMS edge rescue ---
    # device sweep used PRE-merge boxes: flagged rows cover all edges
    # between unmerged boxes; merged rows are rescanned fully.
    r_unsorted = set(int(x) for x in flags_b) | set(merged_rows)
    thr_nms = np.float32(NMS_IOU)
    edges = {}
    for r in r_unsorted:
        p = int(inv[r])
        iou = _iou_row(bs[p], bs)
        hits = np.nonzero((iou >= thr_nms) & (ls == ls[p]))[0]
        for qq in hits:
            qq = int(qq)
            if qq == p:
                continue
            lo, hi = (p, qq) if p < qq else (qq, p)
            edges.setdefault(lo, set()).add(hi)

    # --- sequential suppression scan (only edge nodes matter) ---
    suppressed = ~vs
    keep = np.zeros(NB, dtype=bool)
    if edges:
        for i in range(NB):
            k = not suppressed[i]
            keep[i] = k
            if k and i in edges:
                for b_ in edges[i]:
                    suppressed[b_] = True
    else:
        keep = vs.copy()

    label_map = np.array([2, 1], dtype=np.int32)
    boxes_out = (bs * wh).astype(np.float32)
    labels_out = label_map[np.clip(ls, 0, 1)].astype(np.int32)
    return boxes_out, labels_out, ss.astype(np.float32), keep


# revision 23
# speedup vs baseline: 1.0752x; 1.0222x over previous
"""Ensemble detection fusion (weighted-boxes-fusion match + soft-NMS dedup)
for Trainium2, 8 NeuronCores.

Strategy: the O(N^2) work — greedy-match IoU tests (yolo x frcnn @ 0.8) and
NMS IoU tests (all x all @ 0.95) — runs on-device as ONE merged conservative
*filter* sweep, sharded row-wise across the 8 cores.  Matches/suppressions
at these thresholds are extremely sparse, so the host then *rescues* only
the flagged rows with the exact reference arithmetic (fp32, matching op
order) and resolves the short sequential dependency chains (greedy 'used'
set, NMS suppression scan) on those few rows.

Device test per pair (q, t), fp16 pixel space, split across DVE (ops in
their fast perf modes: tensor_scalar 4x, tensor_tensor 2x) and the
otherwise-idle Scalar engine (relu clamps + sign + count accumulation):
    dxpk = min(TX2,qx2) + min(K-TX1, K-qx1)          # = dx + K     (DVE)
    dypk = min(TY2,qy2) + min(K-TY1, K-qy1)          # = dy + K     (DVE)
    dxk, dyk = relu(dxpk), relu(dypk)                               (ACT)
    v    = dxk*dyk - TS_t                                           (DVE)
    cnt  = sum_j sign(v_j - QS_q)                                   (ACT)
with TS/QS = c*(1-MU)*area, c = thr/(1+thr).  In exact arithmetic
inter >= c*(Aq+At) <=> iou >= thr; the +K (3 px) additive slack covers
fp16 coordinate rounding (~1px absolute on dx/dy) and MU covers relative
rounding, so the device pass set is a strict superset of the exact set.
Per-row pass count is recovered from the sign-sum as P = (cnt + C)/2
(C = swept columns); exact-zero ties only ADD margin-zone false positives.

Work layout: the NMS matrix is symmetric, so rows sweep only
j >= 1024*floor(i/1024) (block upper triangle); rows are assigned to cores
round-robin (i mod 8) so each core holds one 128-row tile per 1024-row
block and the triangle is perfectly balanced.  Every pair (i,j), i<j, is
swept by row i.  For yolo rows (i < 4096) the frcnn column range
[4096:6144) is swept with the LOOSER greedy-match threshold (c_A < c_B),
which simultaneously provides the stage-A candidate set and a superset of
the stage-B edges in that region — stage A costs no extra sweep.
"""

import numpy as np

N1, N2 = 4096, 2048
NB = N1 + N2
CORES = 8
YOLO_W = 0.5
FRCNN_W = 0.5
MATCH_IOU = 0.8
NMS_IOU = 0.95
MU = 0.02  # relative margin on the device filter
KPX = 3.0  # additive pixel slack on overlap widths

QB_PT = NB // CORES // 128  # 6 query tiles per core

# chunk plan: (qtile k, target start, size, area-scale selector)
# sel 'B' -> c_B areas (NMS 0.95), sel 'A' -> c_A areas (match 0.8).
# qtile k of core c holds global rows {1024*k + c + 8*m} and sweeps
# targets j >= 1024*k.
PLAN = [
    # ordered so combos needing only early blob ranges run first, the
    # stage-A (SA-dependent) chunks late, and a short chunk last so the
    # final ACT/drain tail is small
    (0, 0, 1024, "B"),
    (0, 1024, 3072, "B"),
    (1, 1024, 3072, "B"),
    (2, 2048, 2048, "B"),
    (4, 4096, 2048, "B"),
    (5, 5120, 1024, "B"),
    (0, 4096, 2048, "A"),
    (1, 4096, 2048, "A"),
    (2, 4096, 2048, "A"),
    (3, 4096, 2048, "A"),
    (3, 3072, 1024, "B"),
]
# columns swept per qtile k (both selectors)
CB_K = [6144 - 1024 * k for k in range(QB_PT)]

# target blob (fp16): X2[NB] Y2[NB] NX1K[NB] NY1K[NB] SB[NB] SA[N2]
# where NX1K = KPX - x1, NY1K = KPX - y1, SB = cB*(1-MU)*area (all boxes),
# SA = cA*(1-MU)*area (frcnn boxes, for the stage-A columns).
TLEN = 5 * NB + N2

_PROGRAM_CACHE = {}


def _emit_combo(nc, mybir, X2, Y2, NX1K, NY1K, TS, q, nqs, pw, pc, out_ap, chunk):
    f16 = mybir.dt.float16
    f32 = mybir.dt.float32
    Alu = mybir.AluOpType
    Act = mybir.ActivationFunctionType
    nqx1k = q[:, 0:1]
    nqy1k = q[:, 1:2]
    qx2 = q[:, 2:3]
    qy2 = q[:, 3:4]
    # x and y halves packed side by side so the add / relu run as single
    # double-width instructions (same cycles, fewer instructions+sems)
    m1 = pw.tile([128, 2 * chunk], f16, tag="m1")
    nc.vector.tensor_scalar(m1[:, :chunk], X2, qx2, None, Alu.min)
    nc.vector.tensor_scalar(m1[:, chunk:], Y2, qy2, None, Alu.min)
    a1 = pw.tile([128, 2 * chunk], f16, tag="a1")
    nc.vector.tensor_scalar(a1[:, :chunk], NX1K, nqx1k, None, Alu.min)
    nc.vector.tensor_scalar(a1[:, chunk:], NY1K, nqy1k, None, Alu.min)
    dxy = pw.tile([128, 2 * chunk], f16, tag="dp")
    nc.vector.tensor_tensor(dxy[:, :], m1[:, :], a1[:, :], Alu.add)
    dk = pw.tile([128, 2 * chunk], f16, tag="dk")
    nc.scalar.activation(dk[:, :], dxy[:, :], Act.Relu)
    p = pw.tile([128, chunk], f16, tag="p")
    nc.vector.tensor_tensor(p[:, :], dk[:, :chunk], dk[:, chunk:], Alu.mult)
    v = pw.tile([128, chunk], f16, tag="v")
    nc.vector.tensor_tensor(v[:, :], p[:, :], TS, Alu.subtract)
    g = pw.tile([128, chunk], f16, tag="g")
    cnt = pc.tile([128, 1], f32, tag="cnt")
    nc.scalar.activation(
        g[:, :], v[:, :], Act.Sign, bias=nqs, accum_out=cnt[:, :]
    )
    nc.sync.dma_start(out_ap, cnt[:, :])


def _build_program():
    import concourse.bacc as bacc
    import concourse.mybir as mybir
    from concourse import tile

    f16 = mybir.dt.float16
    f32 = mybir.dt.float32
    nc = bacc.Bacc(
        "TRN2", target_bir_lowering=False, debug=False, num_devices=CORES
    )
    qb = nc.dram_tensor("qb", [QB_PT, 128, 6], f32, kind="ExternalInput")
    tbl = nc.dram_tensor("tbl", [1, TLEN], f16, kind="ExternalInput")
    outc = nc.dram_tensor("outc", [len(PLAN), 128, 1], f32, kind="ExternalOutput")

    with tile.TileContext(nc) as tc:
        with (
            tc.tile_pool(name="tgt", bufs=1) as pt,
            tc.tile_pool(name="qs", bufs=8) as pq,
            tc.tile_pool(name="wk", bufs=2) as pw,
            tc.tile_pool(name="ct", bufs=6) as pc,
        ):
            tfull = pt.tile([128, TLEN], f16, tag="tfull")

            # replicate the target blob across all 128 partitions with
            # stride-0-source DMAs on the sync queue (no GpSimd engine
            # involvement -> no shared-SBUF-port contention with DVE).
            # Pieces are emitted on demand just before the first combo
            # that reads them, so each combo's semaphore wait gates on an
            # early cumulative DMA count instead of the whole broadcast.
            done_ranges = set()

            bcast_engines = [nc.sync, nc.scalar]
            bcast_i = [0]

            def bcast(off, n):
                eng = bcast_engines[bcast_i[0] % 2]
                bcast_i[0] += 1
                eng.dma_start(
                    tfull[:, off : off + n],
                    tbl.ap()[0:1, off : off + n].partition_broadcast(128),
                )

            def need_cols(start, size, sel):
                for r in range(start // 1024, (start + size + 1023) // 1024):
                    for b in range(5):  # X2 Y2 NX1K NY1K SB blocks
                        key = (b, r)
                        if key not in done_ranges:
                            done_ranges.add(key)
                            bcast(b * NB + r * 1024, 1024)
                if sel == "A":
                    for r in range(2):
                        key = (5, r)
                        if key not in done_ranges:
                            done_ranges.add(key)
                            bcast(5 * NB + r * 1024, 1024)

            X2 = tfull[:, 0 * NB : 1 * NB]
            Y2 = tfull[:, 1 * NB : 2 * NB]
            NX1K = tfull[:, 2 * NB : 3 * NB]
            NY1K = tfull[:, 3 * NB : 4 * NB]
            SB = tfull[:, 4 * NB : 5 * NB]
            SA = tfull[:, 5 * NB : 5 * NB + N2]

            qtiles = {}
            for idx, (k, start, size, sel) in enumerate(PLAN):
                if k not in qtiles:
                    q = pq.tile([128, 6], f32, tag="q")
                    nc.sync.dma_start(q[:, :], qb.ap()[k, :, :])
                    qtiles[k] = q
                q = qtiles[k]
                need_cols(start, size, sel)
                sl = slice(start, start + size)
                ts_ap = SB[:, sl] if sel == "B" else SA[:, start - N1 : start - N1 + size]
                nqs = q[:, 4:5] if sel == "B" else q[:, 5:6]
                _emit_combo(
                    nc, mybir,
                    X2[:, sl], Y2[:, sl], NX1K[:, sl], NY1K[:, sl], ts_ap,
                    q, nqs, pw, pc, outc.ap()[idx, :, :], size,
                )
    nc.compile()
    return nc


def get_program():
    if "nc" not in _PROGRAM_CACHE:
        _PROGRAM_CACHE["nc"] = _build_program()
    return _PROGRAM_CACHE["nc"]


def _iou_row(box, B):
    # Exact replica of reference _iou_one_vs_many op order (fp32, IEEE).
    x1 = np.maximum(box[0], B[:, 0])
    y1 = np.maximum(box[1], B[:, 1])
    x2 = np.minimum(box[2], B[:, 2])
    y2 = np.minimum(box[3], B[:, 3])
    inter = np.maximum(x2 - x1, np.float32(0.0)) * np.maximum(y2 - y1, np.float32(0.0))
    a1 = (box[2] - box[0]) * (box[3] - box[1])
    a2 = (B[:, 2] - B[:, 0]) * (B[:, 3] - B[:, 1])
    return inter / (a1 + a2 - inter)


def _stage_b_rows(core):
    """Global row indices handled by `core`, tile-major: [k, m] -> row."""
    k = np.arange(QB_PT)[:, None]
    m = np.arange(128)[None, :]
    return 1024 * k + core + 8 * m


def make_device_inputs(pall):
    """pall: pixel-space fp32 box array, yolo rows then frcnn rows."""
    aall = (pall[:, 2] - pall[:, 0]) * (pall[:, 3] - pall[:, 1])
    cA = np.float32((1.0 - MU) * MATCH_IOU / (1.0 + MATCH_IOU))
    cB = np.float32((1.0 - MU) * NMS_IOU / (1.0 + NMS_IOU))
    kpx = np.float32(KPX)

    # query cols: K-x1, K-y1, x2, y2, -cB*A, -cA*A (ACT bias adds before Sign)
    qb_all = np.stack(
        [kpx - pall[:, 0], kpx - pall[:, 1], pall[:, 2], pall[:, 3],
         -cB * aall, -cA * aall], axis=1
    ).astype(np.float32)
    tbl = np.concatenate(
        [pall[:, 2], pall[:, 3], kpx - pall[:, 0], kpx - pall[:, 1],
         cB * aall, cA * aall[N1:]]
    ).astype(np.float16).reshape(1, -1)
    tbl = np.ascontiguousarray(tbl)
    in_maps = []
    for c in range(CORES):
        rows = _stage_b_rows(c).reshape(-1)
        in_maps.append(
            {
                "qb": np.ascontiguousarray(qb_all[rows].reshape(QB_PT, 128, 6)),
                "tbl": tbl,
            }
        )
    return in_maps


def kernel(**inputs):
    yolo_boxes = np.asarray(inputs["yolo_boxes"], dtype=np.float32)
    yolo_scores = np.asarray(inputs["yolo_scores"], dtype=np.float32)
    yolo_labels = np.asarray(inputs["yolo_labels"], dtype=np.int32)
    frcnn_boxes = np.asarray(inputs["frcnn_boxes"], dtype=np.float32)
    frcnn_scores = np.asarray(inputs["frcnn_scores"], dtype=np.float32)
    frcnn_labels = np.asarray(inputs["frcnn_labels"], dtype=np.int32)
    h = float(np.asarray(inputs["h"]))
    w = float(np.asarray(inputs["w"]))

    wh = np.array([w, h, w, h], dtype=np.float32)
    b1 = (yolo_boxes / wh).astype(np.float32)
    b2 = (frcnn_boxes / wh).astype(np.float32)
    s1 = (yolo_scores * np.float32(YOLO_W)).astype(np.float32)
    s2 = (frcnn_scores * np.float32(FRCNN_W)).astype(np.float32)
    l1, l2 = yolo_labels, frcnn_labels

    # --- device filter: 8-core SPMD merged IoU-test sweep ---
    from concourse.bass_utils import run_bass_kernel_spmd

    nc = get_program()
    pall = np.concatenate([yolo_boxes, frcnn_boxes], axis=0)
    in_maps = make_device_inputs(pall)
    import time as _time

    _t0 = _time.time()
    res = run_bass_kernel_spmd(nc, in_maps, core_ids=list(range(CORES)))
    _PROGRAM_CACHE["device_wall_ns"] = int((_time.time() - _t0) * 1e9)
    if getattr(res, "exec_time_ns", None) is not None:
        _PROGRAM_CACHE["exec_time_ns"] = res.exec_time_ns

    # sign-sums -> pass counts: P = (sum + C)/2
    sig_all = np.zeros(NB, dtype=np.float64)   # all chunks (flags_b)
    sig_a = np.zeros(NB, dtype=np.float64)     # cA chunks only (flags_a)
    for c in range(CORES):
        rows = _stage_b_rows(c)  # [QB_PT, 128]
        oc = res.results[c]["outc"].reshape(len(PLAN), 128)
        for idx, (k, _start, _size, sel) in enumerate(PLAN):
            sig_all[rows[k]] += oc[idx]
            if sel == "A":
                sig_a[rows[k]] += oc[idx]

    pass_a = (sig_a[:N1] + N2) * 0.5
    flags_a = np.nonzero(pass_a > 0.5)[0]

    cb = np.zeros(NB)
    for k in range(QB_PT):
        cb[1024 * k : 1024 * (k + 1)] = CB_K[k]
    pass_b = (sig_all + cb) * 0.5
    # every row sweeps itself (diagonal), so > 1 means a real
    # (off-diagonal) near-duplicate candidate
    flags_b = np.nonzero(pass_b > 1.5)[0]

    return _host_resolve(b1, b2, s1, s2, l1, l2, wh, flags_a, flags_b)


def _host_resolve(b1, b2, s1, s2, l1, l2, wh, flags_a, flags_b):
    # --- exact greedy match on candidate rows only ---
    used = np.zeros(N2, dtype=bool)
    mboxes = b1.copy()
    mscores = s1.copy()
    merged_rows = []
    thr_match = np.float32(MATCH_IOU)
    for i in flags_a:
        iou = _iou_row(b1[i], b2)
        iou = np.where((~used) & (l2 == l1[i]), iou, np.float32(0.0))
        j = int(np.argmax(iou))
        if iou[j] >= thr_match:
            tot = s1[i] + s2[j]
            mboxes[i] = (b1[i] * s1[i] + b2[j] * s2[j]) / tot
            mscores[i] = tot
            used[j] = True
            merged_rows.append(int(i))

    boxes_all = np.concatenate([mboxes, b2], axis=0)
    scores_all = np.concatenate([mscores, s2], axis=0)
    labels_all = np.concatenate([l1, l2], axis=0)
    valid = np.concatenate([np.ones(N1, dtype=bool), ~used])

    key = np.where(valid, scores_all, np.float32(-1.0))
    order = np.argsort(-key, kind="stable")
    bs = boxes_all[order]
    ss = scores_all[order]
    ls = labels_all[order]
    vs = valid[order]
    inv = np.empty(NB, dtype=np.int64)
    inv[order] = np.arange(NB)

    # --- exact NMS edge rescue ---
    # device sweep used PRE-merge boxes: flagged rows cover all edges
    # between unmerged boxes; merged rows are rescanned fully.
    r_unsorted = set(int(x) for x in flags_b) | set(merged_rows)
    thr_nms = np.float32(NMS_IOU)
    edges = {}
    for r in r_unsorted:
        p = int(inv[r])
        iou = _iou_row(bs[p], bs)
        hits = np.nonzero((iou >= thr_nms) & (ls == ls[p]))[0]
        for qq in hits:
            qq = int(qq)
            if qq == p:
                continue
            lo, hi = (p, qq) if p < qq else (qq, p)
            edges.setdefault(lo, set()).add(hi)

    # --- sequential suppression scan (only edge nodes matter) ---
    suppressed = ~vs
    keep = np.zeros(NB, dtype=bool)
    if edges:
        for i in range(NB):
            k = not suppressed[i]
            keep[i] = k
            if k and i in edges:
                for b_ in edges[i]:
                    suppressed[b_] = True
    else:
        keep = vs.copy()

    label_map = np.array([2, 1], dtype=np.int32)
    boxes_out = (bs * wh).astype(np.float32)
    labels_out = label_map[np.clip(ls, 0, 1)].astype(np.int32)
    return boxes_out, labels_out, ss.astype(np.float32), keep
